# revision 1
# baseline (speedup 1.0000x reference)
"""Trainium2 Bass kernel for nn_MHA_36584531427723.

Sharding: 8 cores = 2 batches x 4 head-groups (4 heads of 64 dims each per
core). Each core computes its batch's Q/K/V projections restricted to its
head-group's 256 output features, attention for its 4 heads, and a partial
output projection (its 256 rows of Wo^T). The host sums the 4 partials per
batch and adds bo.

Device layout choices (all host-prepped, no on-device transposes):
  - QT/KT = Q[b].T, K[b].T   [1024, 2048] f32  (feature on partitions)
  - projections produce Q_^T/K_^T [256, 2048] (bf16) and V [2048, 4, 65] bf16
    with a ones column at index 64 so the PV matmul also yields the softmax
    denominator row.
  - scores are computed transposed, E^T [k, q], so exp/mask/PV all use
    natural slices; mask is shipped pre-transposed as bf16 0/1.
  - softmax: max-subtraction dropped (|E|<~1 so exp is safe; the reference's
    max shift cancels exactly up to its eps term, relative effect ~1e-11);
    eps dropped (eps/S ~ 1e-11).
"""

import numpy as np
import ml_dtypes

import concourse.bacc as bacc
import concourse.bass as bass  # noqa: F401
import concourse.mybir as mybir
import concourse.tile as tile
from concourse.bass_utils import run_bass_kernel_spmd

B, N, D = 2, 2048, 1024
H = 16
HD = 64
HL = 4  # heads per core
DL = HL * HD  # 256 local features
P = 128
KO = D // P  # 8 contraction chunks for projections
NKC = N // P  # 16 k-token chunks
NQC = N // P
NPAN = 4
PANW = N // NPAN  # 512-wide token panels in the projection phase
SCALE = 1.0 / 32.0  # 1/sqrt(DIM_V)

F32 = mybir.dt.float32
F32R = mybir.dt.float32r
BF16 = mybir.dt.bfloat16
AF = mybir.ActivationFunctionType

def build_nc():
    nc = bacc.Bacc(None, target_bir_lowering=False)
    QT = nc.dram_tensor("qt", (D, N), BF16, kind="ExternalInput")
    KT = nc.dram_tensor("kt", (D, N), BF16, kind="ExternalInput")
    MT = nc.dram_tensor("mt", (N, N), BF16, kind="ExternalInput")
    WQT = nc.dram_tensor("wqt", (D, DL), BF16, kind="ExternalInput")
    WKT = nc.dram_tensor("wkt", (D, DL), BF16, kind="ExternalInput")
    WVT = nc.dram_tensor("wvt", (D, DL), BF16, kind="ExternalInput")
    WOT = nc.dram_tensor("wot", (DL, D), BF16, kind="ExternalInput")
    BQ = nc.dram_tensor("bq", (DL,), F32, kind="ExternalInput")
    BK = nc.dram_tensor("bk", (DL,), F32, kind="ExternalInput")
    BV = nc.dram_tensor("bv", (DL,), F32, kind="ExternalInput")
    OUT = nc.dram_tensor("out", (N, D), F32, kind="ExternalOutput")

    qt_r = QT[:].rearrange("(ko p) q -> p ko q", p=P)
    kt_r = KT[:].rearrange("(ko p) q -> p ko q", p=P)
    mt_r = MT[:].rearrange("(kc p) q -> p kc q", p=P)

    with tile.TileContext(nc) as tc:
        with (
            tc.tile_pool(name="persist", bufs=1) as persist,
            tc.tile_pool(name="otpool", bufs=1) as otpool,
        ):
            # --- persistent tiles ---
            mT = persist.tile([P, NKC, N], BF16)  # 64KB/part
            qT = persist.tile([P, 2, N], BF16, tag="qT")  # Q_^T, 8KB
            kT = persist.tile([P, 2, N], BF16, tag="kT")
            v_sb = persist.tile([P, NKC, HL, HD + 1], BF16, tag="v")
            ones_sb = persist.tile([1, HD], F32, tag="ones")
            nc.vector.memset(ones_sb[:], 1.0)
            bq_sb = persist.tile([P, 2], F32, tag="bq")
            bk_sb = persist.tile([P, 2], F32, tag="bk")
            bv_rep = persist.tile([P, HL, HD], F32, tag="bv")
            wo_sb = persist.tile([P, 2, D], BF16, tag="wo")

            nc.sync.dma_start(out=bq_sb[:], in_=BQ[:].rearrange("(c p) -> p c", p=P))
            nc.sync.dma_start(out=bk_sb[:], in_=BK[:].rearrange("(c p) -> p c", p=P))
            nc.sync.dma_start(
                out=bv_rep[:],
                in_=BV[:].rearrange("(h d) -> h d", h=HL)[None].to_broadcast(
                    (P, HL, HD)
                ),
            )
            for cc in range(2):
                nc.sync.dma_start(
                    out=wo_sb[:, cc, :],
                    in_=WOT[:].rearrange("(cc p) n -> p cc n", p=P)[:, cc, :],
                )
            nc.vector.memset(v_sb[:, :, :, HD : HD + 1], 1.0)

            # ---------------- Phase A: projections ----------------
            with (
                tc.tile_pool(name="wpool", bufs=1) as wpool,
                tc.tile_pool(name="panpool", bufs=2) as panpool,
                tc.tile_pool(name="pjpsum", bufs=4, space="PSUM") as pjpsum,
                tc.tile_pool(name="vpsum", bufs=4, space="PSUM") as vpsum,
            ):
                wq_sb = wpool.tile([P, KO, DL], BF16, tag="wq")
                wk_sb = wpool.tile([P, KO, DL], BF16, tag="wk")
                wv_sb = wpool.tile([P, KO, DL], BF16, tag="wv")
                for w_sb, W in ((wq_sb, WQT), (wk_sb, WKT), (wv_sb, WVT)):
                    nc.sync.dma_start(
                        out=w_sb[:], in_=W[:].rearrange("(ko p) m -> p ko m", p=P)
                    )

                for pan in range(NPAN):
                    qs = slice(pan * PANW, (pan + 1) * PANW)
                    qt_pan = panpool.tile([P, KO, PANW], BF16, tag="qt_pan")
                    kt_pan = panpool.tile([P, KO, PANW], BF16, tag="kt_pan")
                    for ko in range(KO):
                        nc.sync.dma_start(out=qt_pan[:, ko, :], in_=qt_r[:, ko, qs])
                        nc.sync.dma_start(out=kt_pan[:, ko, :], in_=kt_r[:, ko, qs])

                    # Q_^T and K_^T (feature-on-partition), bias fused in evict
                    for pan_in, w_sb, b_sb, dst in (
                        (qt_pan, wq_sb, bq_sb, qT),
                        (kt_pan, wk_sb, bk_sb, kT),
                    ):
                        for dc in range(2):
                            ps = pjpsum.tile([P, PANW], F32, tag="pj")
                            for ko in range(KO):
                                nc.tensor.matmul(
                                    ps[:],
                                    lhsT=(w_sb[:, ko, dc * P : (dc + 1) * P]),
                                    rhs=(pan_in[:, ko, :]),
                                    start=(ko == 0),
                                    stop=(ko == KO - 1),
                                )
                            nc.scalar.activation(
                                out=dst[:, dc, qs],
                                in_=ps[:],
                                func=AF.Identity,
                                bias=b_sb[:, dc : dc + 1],
                                scale=1.0,
                            )

                    # V natural layout (token-on-partition), bias via DVE add
                    for t4 in range(PANW // P):
                        tci = pan * (PANW // P) + t4
                        psv = vpsum.tile([P, DL], F32, tag="pv")
                        for ko in range(KO):
                            nc.tensor.matmul(
                                psv[:],
                                lhsT=(
                                    kt_pan[:, ko, t4 * P : (t4 + 1) * P]
                                ),
                                rhs=(wv_sb[:, ko, :]),
                                start=(ko == 0),
                                stop=(ko == KO - 1),
                            )
                        nc.vector.tensor_add(
                            out=v_sb[:, tci, :, 0:HD],
                            in0=psv[:].rearrange("p (h d) -> p h d", h=HL),
                            in1=bv_rep[:],
                        )

                # mask load last so it fills DMA gaps during phase A
                for kc in range(NKC):
                    nc.sync.dma_start(out=mT[:, kc, :], in_=mt_r[:, kc, :])

            # ---------------- Phase B: attention ----------------
            oT = otpool.tile([P, 2, N], BF16)
            with (
                tc.tile_pool(name="expool", bufs=2) as expool,
                tc.tile_pool(name="srpool", bufs=2) as srpool,
                tc.tile_pool(name="spsum", bufs=2, space="PSUM") as spsum,
                tc.tile_pool(name="opsum", bufs=2, space="PSUM") as opsum,
            ):
                for h in range(HL):
                    dc, po = h // 2, (h % 2) * HD
                    for qg in range(N // 1024):
                        ex = expool.tile([P, NKC, 1024], BF16, tag="ex")
                        for kc in range(NKC):
                            ps = spsum.tile([P, 1024], F32, tag="es")
                            for half in range(2):
                                q0 = qg * 1024 + half * 512
                                nc.tensor.matmul(
                                    ps[:, half * 512 : (half + 1) * 512],
                                    lhsT=kT[po : po + HD, dc, kc * P : (kc + 1) * P],
                                    rhs=qT[po : po + HD, dc, q0 : q0 + 512],
                                    start=True,
                                    stop=True,
                                )
                            nc.scalar.activation(
                                out=ex[:, kc, :], in_=ps[:], func=AF.Exp, scale=SCALE
                            )
                            nc.vector.tensor_mul(
                                out=ex[:, kc, :],
                                in0=ex[:, kc, :],
                                in1=mT[:, kc, qg * 1024 : (qg + 1) * 1024],
                            )
                        for qbh in range(2):
                            pso = opsum.tile([HD + 1, 512], F32, tag="pvo")
                            for kc in range(NKC):
                                nc.tensor.matmul(
                                    pso[:],
                                    lhsT=v_sb[:, kc, h, :],
                                    rhs=ex[:, kc, qbh * 512 : (qbh + 1) * 512],
                                    start=(kc == 0),
                                    stop=(kc == NKC - 1),
                                )
                            s_row = srpool.tile([1, 512], F32, tag="srow")
                            nc.scalar.copy(out=s_row[:], in_=pso[HD : HD + 1, :])
                            srp = opsum.tile([HD, 512], F32, tag="srp")
                            nc.tensor.matmul(
                                srp[:],
                                lhsT=ones_sb[:],
                                rhs=s_row[:],
                                start=True,
                                stop=True,
                            )
                            s_rep = srpool.tile([HD, 512], F32, tag="srep")
                            nc.vector.reciprocal(out=s_rep[:], in_=srp[:])
                            o_tmp = srpool.tile([HD, 512], BF16, tag="otmp")
                            nc.vector.tensor_mul(
                                out=o_tmp[:], in0=pso[0:HD, :], in1=s_rep[:]
                            )
                            q0 = qg * 1024 + qbh * 512
                            nc.sync.dma_start(
                                out=oT[po : po + HD, dc, q0 : q0 + 512], in_=o_tmp[:]
                            )

            # ---------------- Phase C: output projection ----------------
            with (
                tc.tile_pool(name="cout", bufs=3) as cout,
                tc.tile_pool(name="cpsum", bufs=4, space="PSUM") as cpsum,
            ):
                for qc in range(NQC):
                    pss = [
                        cpsum.tile([P, 512], F32, tag="co", name=f"co{i}")
                        for i in range(2)
                    ]
                    for cc in range(2):
                        for nh in range(2):
                            nc.tensor.matmul(
                                pss[nh][:],
                                lhsT=(oT[:, cc, qc * P : (qc + 1) * P]),
                                rhs=(wo_sb[:, cc, nh * 512 : (nh + 1) * 512]),
                                start=(cc == 0),
                                stop=(cc == 1),
                            )
                    o_sb = cout.tile([P, D], F32, tag="osb")
                    for nh in range(2):
                        nc.vector.tensor_copy(
                            out=o_sb[:, nh * 512 : (nh + 1) * 512], in_=pss[nh][:]
                        )
                    nc.sync.dma_start(out=OUT[qc * P : (qc + 1) * P, :], in_=o_sb[:])

    nc.finalize()
    return nc


_NC = None


def _get_nc():
    global _NC
    if _NC is None:
        _NC = build_nc()
    return _NC


def make_in_maps(Q, K, mask, Wq, bq, Wk, bk, Wv, bv, Wo, bo):
    Q = np.asarray(Q, np.float32)
    K = np.asarray(K, np.float32)
    mask = np.asarray(mask)
    Wq = np.asarray(Wq, np.float32)
    Wk = np.asarray(Wk, np.float32)
    Wv = np.asarray(Wv, np.float32)
    Wo = np.asarray(Wo, np.float32)
    qt = [np.ascontiguousarray(Q[b].T).astype(ml_dtypes.bfloat16) for b in range(B)]
    kt = [np.ascontiguousarray(K[b].T).astype(ml_dtypes.bfloat16) for b in range(B)]
    mt = [
        np.ascontiguousarray(mask[b].T).astype(ml_dtypes.bfloat16) for b in range(B)
    ]
    in_maps = []
    for c in range(8):
        b, hg = divmod(c, 4)
        cols = slice(hg * DL, (hg + 1) * DL)
        in_maps.append(
            {
                "qt": qt[b],
                "kt": kt[b],
                "mt": mt[b],
                "wqt": np.ascontiguousarray(Wq[cols, :].T).astype(ml_dtypes.bfloat16),
                "wkt": np.ascontiguousarray(Wk[cols, :].T).astype(ml_dtypes.bfloat16),
                "wvt": np.ascontiguousarray(Wv[cols, :].T).astype(ml_dtypes.bfloat16),
                "wot": np.ascontiguousarray(Wo[:, cols].T).astype(ml_dtypes.bfloat16),
                "bq": np.ascontiguousarray(np.asarray(bq, np.float32)[cols]),
                "bk": np.ascontiguousarray(np.asarray(bk, np.float32)[cols]),
                "bv": np.ascontiguousarray(np.asarray(bv, np.float32)[cols]),
            }
        )
    return in_maps


def assemble(results, bo):
    O = np.zeros((B, N, D), np.float32)
    for c in range(8):
        b = c // 4
        O[b] += results[c]["out"]
    O += np.asarray(bo, np.float32)[None, None, :]
    return O


def kernel(Q, K, mask, Wq, bq, Wk, bk, Wv, bv, Wo, bo):
    nc = _get_nc()
    in_maps = make_in_maps(Q, K, mask, Wq, bq, Wk, bk, Wv, bv, Wo, bo)
    res = run_bass_kernel_spmd(nc, in_maps, core_ids=list(range(8)))
    return assemble(res.results, bo)



# revision 36
# speedup vs baseline: 1.1141x; 1.1141x over previous
"""Trainium2 Bass kernel for nn_MHA_36584531427723.

Sharding: 8 cores = 2 batches x 4 head-groups (4 heads of 64 dims each per
core). Each core computes its batch's Q/K/V projections restricted to its
head-group's 256 output features, attention for its 4 heads, and a partial
output projection (its 256 rows of Wo^T). The host sums the 4 partials per
batch and adds bo.

Device-side structure (v2 — restructured for the timeline cost model):
  - QK scores are computed transposed, E^T [k, q] (k on partitions), 1024-q
    windows per (head, qg); exp on ACT (1024-wide instrs from 2-bank PSUM
    tiles), mask multiply on DVE/GPSIMD (bf16 2x).
  - PV is flipped: out O[q_part, 65_free] (64 dims + ones-column denominator),
    16 kc accumulation steps of only 65 output rows each — half the PE cost of
    the O^T orientation under the free-dim cost model, and the softmax
    denominator lands as a per-partition column (cheap DVE normalize).
  - O is normalized per head into onorm [q, 256], DMA-transposed (xbar) into
    oT [256-feat, q] for the output projection; out-proj streams from PSUM
    straight to DRAM.
  - Optional fp8e4(+DoubleRow) QK: Q_^T/K_^T are evicted as fp8e4 and
    DMA-folded into [32, 2, N] interleaved layout; each QK matmul then
    contracts 2x64 virtual rows and is charged half cost.
  - softmax: max-subtraction dropped (|E/32| < ~0.7 so exp is safe; the
    reference's max shift cancels exactly up to its eps term); eps dropped
    (eps/S ~ 1e-11).
"""

import numpy as np
import ml_dtypes

import concourse.bacc as bacc
import concourse.bass as bass  # noqa: F401
import concourse.mybir as mybir
import concourse.tile as tile
from concourse.bass_utils import run_bass_kernel_spmd

B, N, D = 2, 2048, 1024
H = 16
HD = 64
HL = 4  # heads per core
DL = HL * HD  # 256 local features
P = 128
KO = D // P  # 8 contraction chunks for projections
NKC = N // P  # 16 k-token chunks
NQC = N // P
QG = 1024  # q window width
NQG = N // QG
PAN = 256  # projection panel (tokens)
NPAN = N // PAN
SCALE = 1.0 / 32.0  # 1/sqrt(DIM_V)

F32 = mybir.dt.float32
BF16 = mybir.dt.bfloat16
FP8 = mybir.dt.float8e4
AF = mybir.ActivationFunctionType
ALU = mybir.AluOpType
PM = mybir.MatmulPerfMode

# ---- tuning knobs ----
USE_FP8_QK = False  # fp8e4 + DoubleRow for the QK matmul
QUAD_KCS = (5, 11)  # kc indices whose exp runs on DVE (quad approx)
POOL_MASK_EVERY = 5  # every n-th mask multiply goes to GPSIMD
PV_DELAY = 3  # PV batches trail the exp/mask stream by this many kc
FP8_EVICT_ON_DVE = True  # evict Q/K projections to fp8 on DVE (else ACT)


def build_nc():
    nc = bacc.Bacc(None, target_bir_lowering=False)
    QT = nc.dram_tensor("qt", (D, N), BF16, kind="ExternalInput")
    KT = nc.dram_tensor("kt", (D, N), BF16, kind="ExternalInput")
    MT = nc.dram_tensor("mt", (N, N), BF16, kind="ExternalInput")
    WQT = nc.dram_tensor("wqt", (D, DL), BF16, kind="ExternalInput")
    WKT = nc.dram_tensor("wkt", (D, DL), BF16, kind="ExternalInput")
    WVT = nc.dram_tensor("wvt", (D, DL), BF16, kind="ExternalInput")
    WOT = nc.dram_tensor("wot", (DL, D), BF16, kind="ExternalInput")
    BQ = nc.dram_tensor("bq", (DL,), F32, kind="ExternalInput")
    BK = nc.dram_tensor("bk", (DL,), F32, kind="ExternalInput")
    BV = nc.dram_tensor("bv", (DL,), F32, kind="ExternalInput")
    OUT = nc.dram_tensor("out", (N, D), F32, kind="ExternalOutput")

    qt_r = QT[:].rearrange("(ko p) q -> p ko q", p=P)
    kt_r = KT[:].rearrange("(ko p) q -> p ko q", p=P)
    mt_r = MT[:].rearrange("(kc p) q -> p kc q", p=P)

    with tile.TileContext(nc) as tc:
        with (
            tc.tile_pool(name="persist", bufs=1) as persist,
            tc.tile_pool(name="panpool", bufs=2) as panpool,
            tc.tile_pool(name="rcpool", bufs=4) as rcpool,
            tc.tile_pool(name="qpool", bufs=2) as qpool,
            tc.tile_pool(name="qkps", bufs=2, space="PSUM") as qkps,
            tc.tile_pool(name="pvps", bufs=2, space="PSUM") as pvps,
            tc.tile_pool(name="mmps", bufs=2, space="PSUM") as mmps,
        ):
            # ---------------- persistent tiles ----------------
            mT = persist.tile([P, NKC, N], BF16)  # 64KB/part
            v_sb = persist.tile([P, NKC, HL, HD + 1], BF16, tag="v")
            wq_sb = persist.tile([P, KO, DL], BF16, tag="wq")
            wk_sb = persist.tile([P, KO, DL], BF16, tag="wk")
            wv_sb = persist.tile([P, KO, DL], BF16, tag="wv")
            wo_sb = persist.tile([P, 2, D], BF16, tag="wo")
            bq_sb = persist.tile([P, 2], F32, tag="bq")
            bk_sb = persist.tile([P, 2], F32, tag="bk")
            bv_rep = persist.tile([P, HL, HD], F32, tag="bv")
            onorm = persist.tile([P, 2, NQC, P], BF16, tag="onorm")
            oT = persist.tile([P, 2, N], BF16, tag="oT")
            mex = persist.tile([P, NKC, QG], BF16, tag="mex")
            if USE_FP8_QK:
                q8 = persist.tile([P, 2, N], FP8, tag="q8")
                k8 = persist.tile([P, 2, N], FP8, tag="k8")
                q8t = persist.tile([P, 2, N], FP8, tag="q8t")
                k8t = persist.tile([P, 2, N], FP8, tag="k8t")
            else:
                qT = persist.tile([P, 2, N], BF16, tag="qT")
                kT = persist.tile([P, 2, N], BF16, tag="kT")

            # ---------------- weight/bias DMAs ----------------
            # Two HWDGE queues: SP carries the K-side + masks, the (idle at
            # prefix time) ACT engine carries the Q-side + V/O weights.
            nc.sync.dma_start(
                out=wk_sb[:], in_=WKT[:].rearrange("(ko p) m -> p ko m", p=P)
            )
            nc.sync.dma_start(
                out=wq_sb[:], in_=WQT[:].rearrange("(ko p) m -> p ko m", p=P)
            )
            def bias_dmas():
                nc.sync.dma_start(
                    out=bk_sb[:], in_=BK[:].rearrange("(c p) -> p c", p=P)
                )
                nc.sync.dma_start(
                    out=bq_sb[:], in_=BQ[:].rearrange("(c p) -> p c", p=P)
                )
                nc.sync.dma_start(
                    out=bv_rep[:],
                    in_=BV[:].rearrange("(h d) -> h d", h=HL)[None].to_broadcast(
                        (P, HL, HD)
                    ),
                )

            nc.vector.memset(v_sb[:, :, :, HD : HD + 1], 1.0)

            def late_weight_dmas():
                nc.sync.dma_start(
                    out=wv_sb[:], in_=WVT[:].rearrange("(ko p) m -> p ko m", p=P)
                )
                for cc in range(2):
                    nc.sync.dma_start(
                        out=wo_sb[:, cc, :],
                        in_=WOT[:].rearrange("(cc p) n -> p cc n", p=P)[:, cc, :],
                    )

            # ---------------- helpers ----------------
            def proj_group(which, pan, dc):
                """Project a 256-token panel of Q^T or K^T for head-pair dc."""
                w_sb, b_sb = (wq_sb, bq_sb) if which == "q" else (wk_sb, bk_sb)
                pan_in = q_panels[pan] if which == "q" else k_panels[pan]
                ps = mmps.tile([P, PAN], F32, tag="mm", name=f"pj{which}{pan}{dc}")
                for ko in range(KO):
                    nc.tensor.matmul(
                        ps[:],
                        lhsT=w_sb[:, ko, dc * P : (dc + 1) * P],
                        rhs=pan_in[:, ko, :],
                        start=(ko == 0),
                        stop=(ko == KO - 1),
                    )
                qs = slice(pan * PAN, (pan + 1) * PAN)
                if USE_FP8_QK:
                    dst = (q8t if which == "q" else k8t)[:, dc, qs]
                    if FP8_EVICT_ON_DVE:
                        nc.vector.tensor_scalar(
                            out=dst,
                            in0=ps[:],
                            scalar1=b_sb[:, dc : dc + 1],
                            scalar2=None,
                            op0=ALU.add,
                        )
                    else:
                        nc.scalar.activation(
                            out=dst,
                            in_=ps[:],
                            func=AF.Identity,
                            bias=b_sb[:, dc : dc + 1],
                            scale=1.0,
                        )
                else:
                    dst = (qT if which == "q" else kT)[:, dc, qs]
                    nc.vector.tensor_scalar(
                        out=dst,
                        in0=ps[:],
                        scalar1=b_sb[:, dc : dc + 1],
                        scalar2=None,
                        op0=ALU.add,
                    )

            def v_group(pan, t2):
                """V projection for 128 tokens, all heads (natural layout)."""
                tc_i = pan * 2 + t2
                pan_in = k_panels[pan]
                psv = mmps.tile([P, DL], F32, tag="mm", name=f"pv{tc_i}")
                for ko in range(KO):
                    nc.tensor.matmul(
                        psv[:],
                        lhsT=pan_in[:, ko, t2 * P : (t2 + 1) * P],
                        rhs=wv_sb[:, ko, :],
                        start=(ko == 0),
                        stop=(ko == KO - 1),
                    )
                nc.vector.tensor_add(
                    out=v_sb[:, tc_i, :, 0:HD],
                    in0=psv[:].rearrange("p (h d) -> p h d", h=HL),
                    in1=bv_rep[:],
                )

            def fold_fp8(which, heads, t0, t1):
                """DMA-fold [64-feat, tokens t0:t1] fp8 rows into the
                [32, 2, N] DoubleRow layout, head h at partition 32h."""
                srcs, dsts = (q8t, q8) if which == "q" else (k8t, k8)
                for h in heads:
                    dc, po = h // 2, (h % 2) * HD
                    s = srcs[po : po + HD, dc, t0:t1].rearrange(
                        "(i p) q -> p i q", i=2
                    )
                    nc.sync.dma_start(
                        out=dsts[32 * h : 32 * h + 32, :, t0:t1], in_=s
                    )

            def out_proj(qc, use_act=False):
                osb = qpool.tile([P, D], F32, tag="osb", bufs=4, name=f"ob{qc}")
                for nh in range(2):
                    pool_ = qkps if (use_act and nh == 1) else mmps
                    tag_ = "qk" if (use_act and nh == 1) else "mm"
                    pso = pool_.tile([P, 512], F32, tag=tag_, name=f"op{qc}{nh}")
                    for cc in range(2):
                        nc.tensor.matmul(
                            pso[:],
                            lhsT=oT[:, cc, qc * P : (qc + 1) * P],
                            rhs=wo_sb[:, cc, nh * 512 : (nh + 1) * 512],
                            start=(cc == 0),
                            stop=(cc == 1),
                        )
                    dst = osb[:, nh * 512 : (nh + 1) * 512]
                    if use_act and nh == 1:
                        nc.scalar.copy(out=dst, in_=pso[:])
                    else:
                        nc.vector.tensor_copy(out=dst, in_=pso[:])
                nc.sync.dma_start(out=OUT[qc * P : (qc + 1) * P, :], in_=osb[:])

            bg_queue = []

            def drain_bg(k=1):
                for _ in range(k):
                    if bg_queue:
                        bg_queue.pop(0)()

            # ---------------- prefix: input panels + dc0 proj ----------------
            q_panels = {}
            k_panels = {}

            def load_panel(which, pan, eng=None):
                nbufs = 3 if (USE_FP8_QK and which == "k") else 2
                t = panpool.tile(
                    [P, KO, PAN],
                    BF16,
                    tag=f"{which}pan",
                    bufs=nbufs,
                    name=f"{which}p{pan}",
                )
                src = qt_r if which == "q" else kt_r
                (eng or nc.sync).dma_start(
                    out=t[:], in_=src[:, :, pan * PAN : (pan + 1) * PAN]
                )
                (q_panels if which == "q" else k_panels)[pan] = t

            def load_mask(kc, half):
                nc.sync.dma_start(
                    out=mT[:, kc, half * QG : (half + 1) * QG],
                    in_=mt_r[:, kc, half * QG : (half + 1) * QG],
                )

            # Minimal prefix: only what the first QK/PV of window (0,0) needs.
            # The rest of the dc0/V projections are woven into window 0 via
            # per-kc hooks, several iterations ahead of their consumers.
            bias_dmas()
            # scalar-AP operands (tensor_scalar scalar1) carry no dependency
            # edge; this DVE self-copy fences all later DVE evictions behind
            # the bias DMAs (DVE executes in order).
            nc.vector.tensor_copy(out=bk_sb[:], in_=bk_sb[:])
            nc.vector.tensor_copy(out=bq_sb[:], in_=bq_sb[:])
            load_panel("k", 0)
            load_panel("q", 0)
            load_panel("q", 1)
            proj_group("k", 0, 0)
            proj_group("q", 0, 0)
            load_panel("q", 2)
            proj_group("q", 1, 0)
            load_panel("k", 1)
            proj_group("q", 2, 0)
            load_panel("q", 3)
            late_weight_dmas()  # wv/wo
            proj_group("k", 1, 0)
            proj_group("q", 3, 0)
            v_group(0, 0)
            v_group(0, 1)
            load_panel("k", 2)
            if USE_FP8_QK:
                proj_group("k", 2, 0)
                v_group(1, 0)
                v_group(1, 1)
                load_panel("k", 3)
                proj_group("k", 3, 0)
                fold_fp8("q", (0, 1), 0, 1024)
                fold_fp8("k", (0, 1), 0, 1024)
            for kc in range(4):
                load_mask(kc, 0)

            # window-0 hooks: stream the remaining dc0 K / V projections and
            # the mask halves just ahead of their consumers. Ordering rule: a
            # panel load must be emitted after every consumer of the tile
            # previously occupying its pool slot.
            w0_hooks = {}

            def _add_hook(kc, fn):
                w0_hooks.setdefault(kc, []).append(fn)

            if not USE_FP8_QK:
                for j in range(6):  # K panels 2..7 at kc=2j, used kc=2j+4
                    _add_hook(2 * j, lambda p=j + 2: proj_group("k", p, 0))
                    _add_hook(2 * j + 1, lambda p=j + 1: v_group(p, 0))
                    _add_hook(2 * j + 1, lambda p=j + 1: v_group(p, 1))
                    if j < 5:  # panels 3..7; load after slot's last reader
                        _add_hook(2 * j + 1, lambda p=j + 3: load_panel("k", p))
                _add_hook(13, lambda: v_group(7, 0))
                _add_hook(13, lambda: v_group(7, 1))
            else:
                # K panels 4..7: load at even hooks, project 2 hooks later;
                # folds trail the evictions ahead of the QK kc that needs them.
                _add_hook(0, lambda: load_panel("k", 4))
                for j in range(4):
                    _add_hook(2 * j + 2, lambda p=j + 4: proj_group("k", p, 0))
                    if j < 3:
                        _add_hook(
                            2 * j + 3, lambda p=j + 5: load_panel("k", p)
                        )
                for j in range(6):  # V panels 2..7
                    _add_hook(2 * j + 1, lambda p=j + 2: v_group(p, 0))
                    _add_hook(2 * j + 1, lambda p=j + 2: v_group(p, 1))
                _add_hook(6, lambda: fold_fp8("k", (0, 1), 1024, 1536))
                _add_hook(10, lambda: fold_fp8("k", (0, 1), 1536, 2048))
                _add_hook(11, lambda: fold_fp8("q", (0, 1), 1024, 2048))
            for i in range(4):  # q panels 4..7 for qg1
                _add_hook(2 * i + 1, lambda p=4 + i: load_panel("q", p))
                _add_hook(2 * i + 3, lambda p=4 + i: proj_group("q", p, 0))
            for kc in range(12):  # qg0 mask rows 4..15
                _add_hook(kc, lambda r=kc + 4: load_mask(r, 0))
            for kc in range(8, 16):  # qg1 mask halves during late window 0
                _add_hook(kc, lambda r=2 * (kc - 8): load_mask(r, 1))
                _add_hook(kc, lambda r=2 * (kc - 8) + 1: load_mask(r, 1))

            # background for windows (0,1)..(1,1): dc1 projections
            bg_queue.append(lambda: load_panel("k", 0))
            bg_queue.append(lambda: load_panel("q", 0))
            bg_queue.append(lambda: load_panel("k", 1))
            bg_queue.append(lambda: load_panel("q", 1))

            def dc1_unit(which, pan):
                proj_group(which, pan, 1)
                if pan + 2 < NPAN:
                    load_panel(which, pan + 2)

            for pan in range(NPAN):
                bg_queue.append(lambda p=pan: dc1_unit("k", p))
                bg_queue.append(lambda p=pan: dc1_unit("q", p))
                if USE_FP8_QK and pan == 3:
                    bg_queue.append(lambda: fold_fp8("k", (2, 3), 0, 1024))
                    bg_queue.append(lambda: fold_fp8("q", (2, 3), 0, 1024))
            if USE_FP8_QK:
                bg_queue.append(lambda: fold_fp8("k", (2, 3), 1024, 2048))
                bg_queue.append(lambda: fold_fp8("q", (2, 3), 1024, 2048))

            # ---------------- attention windows ----------------
            mask_ctr = [0]

            def window(h, qg, bg_per_kc, carry, hooks):
                dc, po = h // 2, (h % 2) * HD
                q0 = qg * QG
                pv = [
                    pvps.tile([P, 4, HD + 1], F32, tag="pv", name=f"pv{h}{qg}{i}")
                    for i in range(2)
                ]

                def pv_batch(kc):
                    # one accumulation group per PSUM bank: start once, stop
                    # once; per-element has_written handles first-write-vs-
                    # accumulate across the four qi sub-regions.
                    for qh in range(2):
                        for qi in range(4):
                            nc.tensor.matmul(
                                pv[qh][:, qi, :],
                                lhsT=mex[
                                    :, kc, (qh * 4 + qi) * P : (qh * 4 + qi + 1) * P
                                ],
                                rhs=v_sb[:, kc, h, :],
                                start=(kc == 0 and qi == 0),
                                stop=(kc == NKC - 1 and qi == 3),
                            )

                def norms_qh(qh):
                    def _n():
                        rc = rcpool.tile([P, 4], F32, tag="rc", name=f"rc{h}{qg}{qh}")
                        nc.vector.reciprocal(out=rc[:], in_=pv[qh][:, :, HD])
                        for qi in range(4):
                            qc = qg * 8 + qh * 4 + qi
                            nc.vector.tensor_scalar(
                                out=onorm[:, dc2(h), qc, hcol(h)],
                                in0=pv[qh][:, qi, 0:HD],
                                scalar1=rc[:, qi : qi + 1],
                                scalar2=None,
                                op0=ALU.mult,
                            )

                    return _n

                for kc in range(NKC):
                    if kc < len(carry):
                        carry[kc]()
                    for fn in hooks.get(kc, ()):
                        fn()
                    ps = qkps.tile([P, QG], F32, tag="qk", name=f"qk{h}{qg}{kc}")
                    for half in range(2):
                        hs = slice(half * 512, (half + 1) * 512)
                        if USE_FP8_QK:
                            nc.tensor.matmul(
                                ps[:, hs],
                                lhsT=k8[
                                    32 * h : 32 * h + 32, :, kc * P : (kc + 1) * P
                                ],
                                rhs=q8[
                                    32 * h : 32 * h + 32,
                                    :,
                                    q0 + half * 512 : q0 + (half + 1) * 512,
                                ],
                                start=True,
                                stop=True,
                                perf_mode=PM.DoubleRow,
                                tile_position=(32 * h, 0),
                            )
                        else:
                            nc.tensor.matmul(
                                ps[:, hs],
                                lhsT=kT[po : po + HD, dc, kc * P : (kc + 1) * P],
                                rhs=qT[
                                    po : po + HD,
                                    dc,
                                    q0 + half * 512 : q0 + (half + 1) * 512,
                                ],
                                start=True,
                                stop=True,
                            )
                    if kc in QUAD_KCS:
                        # exp(x) ~= ((E*(E + 4/S) * S^2/8) + 1)^2 on DVE
                        u = qpool.tile([P, QG], BF16, tag="u", name=f"u{h}{qg}{kc}")
                        t2 = qpool.tile([P, QG], BF16, tag="t2", name=f"t{h}{qg}{kc}")
                        nc.vector.scalar_tensor_tensor(
                            out=u[:],
                            in0=ps[:],
                            scalar=4.0 / SCALE,
                            in1=ps[:],
                            op0=ALU.add,
                            op1=ALU.mult,
                        )
                        nc.vector.tensor_scalar(
                            out=t2[:],
                            in0=u[:],
                            scalar1=SCALE * SCALE / 8.0,
                            scalar2=1.0,
                            op0=ALU.mult,
                            op1=ALU.add,
                        )
                        nc.vector.tensor_mul(out=mex[:, kc, :], in0=t2[:], in1=t2[:])
                    else:
                        nc.scalar.activation(
                            out=mex[:, kc, :], in_=ps[:], func=AF.Exp, scale=SCALE
                        )
                    mask_ctr[0] += 1
                    eng = (
                        nc.gpsimd
                        if (POOL_MASK_EVERY and mask_ctr[0] % POOL_MASK_EVERY == 0)
                        else nc.vector
                    )
                    eng.tensor_mul(
                        out=mex[:, kc, :],
                        in0=mex[:, kc, :],
                        in1=mT[:, kc, q0 : q0 + QG],
                    )
                    # PV trails so it never head-blocks the PE queue
                    if kc >= PV_DELAY:
                        pv_batch(kc - PV_DELAY)
                    if bg_per_kc and kc % 4 == 3:
                        drain_bg(bg_per_kc)
                # last two PV batches + normalization are carried into the
                # next window's first iterations (frees this window's tail)
                def _pv_tail():
                    for kc in range(NKC - PV_DELAY, NKC):
                        pv_batch(kc)

                return {"pv_tail": _pv_tail, "norms": [norms_qh(0), norms_qh(1)]}

            def dc2(h):
                return h // 2

            def hcol(h):
                return slice((h % 2) * HD, (h % 2) * HD + HD)

            def transpose_qg(qg, dc):
                # batched xbar transpose: out[p, j, f] = in[f, j*128 + p]
                nc.sync.dma_start_transpose(
                    out=oT[:, dc, qg * QG : (qg + 1) * QG].rearrange(
                        "p (j f) -> p j f", j=8
                    ),
                    in_=onorm[:, dc, qg * 8 : (qg + 1) * 8, :],
                )

            def transpose_half(qg, dc, qh):
                q0 = qg * QG + qh * 512
                nc.sync.dma_start_transpose(
                    out=oT[:, dc, q0 : q0 + 512].rearrange("p (j f) -> p j f", j=4),
                    in_=onorm[:, dc, qg * 8 + qh * 4 : qg * 8 + qh * 4 + 4, :],
                )

            last_carry = [None]

            carry = []

            def run_window(h, qg, bg_per_kc, hooks=None):
                nonlocal carry
                c = window(h, qg, bg_per_kc, carry, hooks or {})

                def _norms_both():
                    c["norms"][0]()
                    c["norms"][1]()

                last_carry[0] = c
                carry = [c["pv_tail"], _norms_both]

            # pair 0 (heads 0,1); window 0 carries the rest of the dc0/V
            # projections via hooks, later windows drain dc1 projections
            run_window(0, 0, 0, hooks=w0_hooks)
            run_window(0, 1, 2)
            run_window(1, 0, 2)
            run_window(1, 1, 2)
            assert not bg_queue, f"{len(bg_queue)} undrained bg units"
            # pair 1 (heads 2,3): out-proj trails head 3
            run_window(2, 0, 0)
            transpose_qg(0, 0)
            transpose_qg(1, 0)
            run_window(2, 1, 0)
            run_window(3, 0, 0)
            for qc in range(8):
                bg_queue.append(lambda c=qc: out_proj(c))
            run_window(3, 1, 2, hooks={2: [lambda: transpose_qg(0, 1)]})
            c = last_carry[0]
            c["pv_tail"]()
            c["norms"][0]()
            transpose_half(1, 1, 0)
            for qc in range(8, 12):
                out_proj(qc, use_act=True)
            c["norms"][1]()
            transpose_half(1, 1, 1)
            for qc in range(12, 16):
                out_proj(qc, use_act=True)

    nc.finalize()
    return nc


_NC = None


def _get_nc():
    global _NC
    if _NC is None:
        _NC = build_nc()
    return _NC


def make_in_maps(Q, K, mask, Wq, bq, Wk, bk, Wv, bv, Wo, bo):
    Q = np.asarray(Q, np.float32)
    K = np.asarray(K, np.float32)
    mask = np.asarray(mask)
    Wq = np.asarray(Wq, np.float32)
    Wk = np.asarray(Wk, np.float32)
    Wv = np.asarray(Wv, np.float32)
    Wo = np.asarray(Wo, np.float32)
    qt = [np.ascontiguousarray(Q[b].T).astype(ml_dtypes.bfloat16) for b in range(B)]
    kt = [np.ascontiguousarray(K[b].T).astype(ml_dtypes.bfloat16) for b in range(B)]
    mt = [
        np.ascontiguousarray(mask[b].T).astype(ml_dtypes.bfloat16) for b in range(B)
    ]
    in_maps = []
    for c in range(8):
        b, hg = divmod(c, 4)
        cols = slice(hg * DL, (hg + 1) * DL)
        in_maps.append(
            {
                "qt": qt[b],
                "kt": kt[b],
                "mt": mt[b],
                "wqt": np.ascontiguousarray(Wq[cols, :].T).astype(ml_dtypes.bfloat16),
                "wkt": np.ascontiguousarray(Wk[cols, :].T).astype(ml_dtypes.bfloat16),
                "wvt": np.ascontiguousarray(Wv[cols, :].T).astype(ml_dtypes.bfloat16),
                "wot": np.ascontiguousarray(Wo[:, cols].T).astype(ml_dtypes.bfloat16),
                "bq": np.ascontiguousarray(np.asarray(bq, np.float32)[cols]),
                "bk": np.ascontiguousarray(np.asarray(bk, np.float32)[cols]),
                "bv": np.ascontiguousarray(np.asarray(bv, np.float32)[cols]),
            }
        )
    return in_maps


def assemble(results, bo):
    O = np.zeros((B, N, D), np.float32)
    for c in range(8):
        b = c // 4
        O[b] += results[c]["out"]
    O += np.asarray(bo, np.float32)[None, None, :]
    return O


def kernel(Q, K, mask, Wq, bq, Wk, bk, Wv, bv, Wo, bo):
    nc = _get_nc()
    in_maps = make_in_maps(Q, K, mask, Wq, bq, Wk, bk, Wv, bv, Wo, bo)
    res = run_bass_kernel_spmd(nc, in_maps, core_ids=list(range(8)))
    return assemble(res.results, bo)


# revision 47
# speedup vs baseline: 1.1183x; 1.0038x over previous
"""Trainium2 Bass kernel for nn_MHA_36584531427723.

Sharding: 8 cores = 2 batches x 4 head-groups (4 heads of 64 dims each per
core). Each core computes its batch's Q/K/V projections restricted to its
head-group's 256 output features, attention for its 4 heads, and a partial
output projection (its 256 rows of Wo^T). The host sums the 4 partials per
batch and adds bo.

Device-side structure (v2 — restructured for the timeline cost model):
  - QK scores are computed transposed, E^T [k, q] (k on partitions), 1024-q
    windows per (head, qg); exp on ACT (1024-wide instrs from 2-bank PSUM
    tiles), mask multiply on DVE/GPSIMD (bf16 2x).
  - PV is flipped: out O[q_part, 65_free] (64 dims + ones-column denominator),
    16 kc accumulation steps of only 65 output rows each — half the PE cost of
    the O^T orientation under the free-dim cost model, and the softmax
    denominator lands as a per-partition column (cheap DVE normalize).
  - O is normalized per head into onorm [q, 256], DMA-transposed (xbar) into
    oT [256-feat, q] for the output projection; out-proj streams from PSUM
    straight to DRAM.
  - Optional fp8e4(+DoubleRow) QK: Q_^T/K_^T are evicted as fp8e4 and
    DMA-folded into [32, 2, N] interleaved layout; each QK matmul then
    contracts 2x64 virtual rows and is charged half cost.
  - softmax: max-subtraction dropped (|E/32| < ~0.7 so exp is safe; the
    reference's max shift cancels exactly up to its eps term); eps dropped
    (eps/S ~ 1e-11).
"""

import numpy as np
import ml_dtypes

import concourse.bacc as bacc
import concourse.bass as bass  # noqa: F401
import concourse.mybir as mybir
import concourse.tile as tile
from concourse.bass_utils import run_bass_kernel_spmd

B, N, D = 2, 2048, 1024
H = 16
HD = 64
HL = 4  # heads per core
DL = HL * HD  # 256 local features
P = 128
KO = D // P  # 8 contraction chunks for projections
NKC = N // P  # 16 k-token chunks
NQC = N // P
QG = 1024  # q window width
NQG = N // QG
PAN = 256  # projection panel (tokens)
NPAN = N // PAN
SCALE = 1.0 / 32.0  # 1/sqrt(DIM_V)

F32 = mybir.dt.float32
BF16 = mybir.dt.bfloat16
FP8 = mybir.dt.float8e4
AF = mybir.ActivationFunctionType
ALU = mybir.AluOpType
PM = mybir.MatmulPerfMode

# ---- tuning knobs ----
USE_FP8_QK = False  # fp8e4 + DoubleRow for the QK matmul
QUAD_KCS = ()  # kc indices whose exp runs on DVE (quad approx)
POOL_MASK_EVERY = 0  # every n-th mask multiply goes to GPSIMD
PV_DELAY = 2  # PV batches trail the exp/mask stream by this many kc
FP8_EVICT_ON_DVE = True  # evict Q/K projections to fp8 on DVE (else ACT)


def build_nc():
    nc = bacc.Bacc(None, target_bir_lowering=False)
    QT = nc.dram_tensor("qt", (D, N), BF16, kind="ExternalInput")
    KT = nc.dram_tensor("kt", (D, N), BF16, kind="ExternalInput")
    MT = nc.dram_tensor("mt", (N, N), BF16, kind="ExternalInput")
    WQT = nc.dram_tensor("wqt", (D, DL), BF16, kind="ExternalInput")
    WKT = nc.dram_tensor("wkt", (D, DL), BF16, kind="ExternalInput")
    WVT = nc.dram_tensor("wvt", (D, DL), BF16, kind="ExternalInput")
    WOT = nc.dram_tensor("wot", (DL, D), BF16, kind="ExternalInput")
    BQ = nc.dram_tensor("bq", (DL,), F32, kind="ExternalInput")
    BK = nc.dram_tensor("bk", (DL,), F32, kind="ExternalInput")
    BV = nc.dram_tensor("bv", (DL,), F32, kind="ExternalInput")
    OUT = nc.dram_tensor("out", (N, D), F32, kind="ExternalOutput")

    qt_r = QT[:].rearrange("(ko p) q -> p ko q", p=P)
    kt_r = KT[:].rearrange("(ko p) q -> p ko q", p=P)
    mt_r = MT[:].rearrange("(kc p) q -> p kc q", p=P)

    with tile.TileContext(nc) as tc:
        with (
            tc.tile_pool(name="persist", bufs=1) as persist,
            tc.tile_pool(name="panpool", bufs=2) as panpool,
            tc.tile_pool(name="rcpool", bufs=4) as rcpool,
            tc.tile_pool(name="qpool", bufs=2) as qpool,
            tc.tile_pool(name="qkps", bufs=2, space="PSUM") as qkps,
            tc.tile_pool(name="pvps", bufs=2, space="PSUM") as pvps,
            tc.tile_pool(name="mmps", bufs=2, space="PSUM") as mmps,
        ):
            # ---------------- persistent tiles ----------------
            mT = persist.tile([P, NKC, N], BF16)  # 64KB/part
            v_sb = persist.tile([P, NKC, HL, HD + 1], BF16, tag="v")
            wq_sb = persist.tile([P, KO, DL], BF16, tag="wq")
            wk_sb = persist.tile([P, KO, DL], BF16, tag="wk")
            wv_sb = persist.tile([P, KO, DL], BF16, tag="wv")
            wo_sb = persist.tile([P, 2, D], BF16, tag="wo")
            bq_sb = persist.tile([P, 2], F32, tag="bq")
            bk_sb = persist.tile([P, 2], F32, tag="bk")
            bv_rep = persist.tile([P, HL, HD], F32, tag="bv")
            onorm = persist.tile([P, 2, NQC, P], BF16, tag="onorm")
            oT = persist.tile([P, 2, N], BF16, tag="oT")
            mex = persist.tile([P, NKC, QG], BF16, tag="mex")
            if USE_FP8_QK:
                q8 = persist.tile([P, 2, N], FP8, tag="q8")
                k8 = persist.tile([P, 2, N], FP8, tag="k8")
                q8t = persist.tile([P, 2, N], FP8, tag="q8t")
                k8t = persist.tile([P, 2, N], FP8, tag="k8t")
            else:
                qT = persist.tile([P, 2, N], BF16, tag="qT")
                kT = persist.tile([P, 2, N], BF16, tag="kT")

            # ---------------- weight/bias DMAs ----------------
            # Two HWDGE queues: SP carries the K-side + masks, the (idle at
            # prefix time) ACT engine carries the Q-side + V/O weights.
            nc.sync.dma_start(
                out=wk_sb[:], in_=WKT[:].rearrange("(ko p) m -> p ko m", p=P)
            )
            nc.sync.dma_start(
                out=wq_sb[:], in_=WQT[:].rearrange("(ko p) m -> p ko m", p=P)
            )
            def bias_dmas():
                nc.sync.dma_start(
                    out=bk_sb[:], in_=BK[:].rearrange("(c p) -> p c", p=P)
                )
                nc.sync.dma_start(
                    out=bq_sb[:], in_=BQ[:].rearrange("(c p) -> p c", p=P)
                )
                nc.sync.dma_start(
                    out=bv_rep[:],
                    in_=BV[:].rearrange("(h d) -> h d", h=HL)[None].to_broadcast(
                        (P, HL, HD)
                    ),
                )

            nc.vector.memset(v_sb[:, :, :, HD : HD + 1], 1.0)

            def late_weight_dmas():
                nc.sync.dma_start(
                    out=wv_sb[:], in_=WVT[:].rearrange("(ko p) m -> p ko m", p=P)
                )
                for cc in range(2):
                    nc.sync.dma_start(
                        out=wo_sb[:, cc, :],
                        in_=WOT[:].rearrange("(cc p) n -> p cc n", p=P)[:, cc, :],
                    )

            # ---------------- helpers ----------------
            def proj_group(which, pan, dc):
                """Project a 256-token panel of Q^T or K^T for head-pair dc."""
                w_sb, b_sb = (wq_sb, bq_sb) if which == "q" else (wk_sb, bk_sb)
                pan_in = q_panels[pan] if which == "q" else k_panels[pan]
                ps = mmps.tile([P, PAN], F32, tag="mm", name=f"pj{which}{pan}{dc}")
                for ko in range(KO):
                    nc.tensor.matmul(
                        ps[:],
                        lhsT=w_sb[:, ko, dc * P : (dc + 1) * P],
                        rhs=pan_in[:, ko, :],
                        start=(ko == 0),
                        stop=(ko == KO - 1),
                    )
                qs = slice(pan * PAN, (pan + 1) * PAN)
                if USE_FP8_QK:
                    dst = (q8t if which == "q" else k8t)[:, dc, qs]
                    if FP8_EVICT_ON_DVE:
                        nc.vector.tensor_scalar(
                            out=dst,
                            in0=ps[:],
                            scalar1=b_sb[:, dc : dc + 1],
                            scalar2=None,
                            op0=ALU.add,
                        )
                    else:
                        nc.scalar.activation(
                            out=dst,
                            in_=ps[:],
                            func=AF.Identity,
                            bias=b_sb[:, dc : dc + 1],
                            scale=1.0,
                        )
                else:
                    dst = (qT if which == "q" else kT)[:, dc, qs]
                    nc.vector.tensor_scalar(
                        out=dst,
                        in0=ps[:],
                        scalar1=b_sb[:, dc : dc + 1],
                        scalar2=None,
                        op0=ALU.add,
                    )

            def v_group(pan, t2):
                """V projection for 128 tokens, all heads (natural layout)."""
                tc_i = pan * 2 + t2
                pan_in = k_panels[pan]
                psv = mmps.tile([P, DL], F32, tag="mm", name=f"pv{tc_i}")
                for ko in range(KO):
                    nc.tensor.matmul(
                        psv[:],
                        lhsT=pan_in[:, ko, t2 * P : (t2 + 1) * P],
                        rhs=wv_sb[:, ko, :],
                        start=(ko == 0),
                        stop=(ko == KO - 1),
                    )
                nc.vector.tensor_add(
                    out=v_sb[:, tc_i, :, 0:HD],
                    in0=psv[:].rearrange("p (h d) -> p h d", h=HL),
                    in1=bv_rep[:],
                )

            def fold_fp8(which, heads, t0, t1):
                """DMA-fold [64-feat, tokens t0:t1] fp8 rows into the
                [32, 2, N] DoubleRow layout, head h at partition 32h."""
                srcs, dsts = (q8t, q8) if which == "q" else (k8t, k8)
                for h in heads:
                    dc, po = h // 2, (h % 2) * HD
                    s = srcs[po : po + HD, dc, t0:t1].rearrange(
                        "(i p) q -> p i q", i=2
                    )
                    nc.sync.dma_start(
                        out=dsts[32 * h : 32 * h + 32, :, t0:t1], in_=s
                    )

            def out_proj(qc, use_act=False):
                osb = qpool.tile([P, D], F32, tag="osb", bufs=4, name=f"ob{qc}")
                for nh in range(2):
                    pool_ = qkps if (use_act and nh == 1) else mmps
                    tag_ = "qk" if (use_act and nh == 1) else "mm"
                    pso = pool_.tile([P, 512], F32, tag=tag_, name=f"op{qc}{nh}")
                    for cc in range(2):
                        nc.tensor.matmul(
                            pso[:],
                            lhsT=oT[:, cc, qc * P : (qc + 1) * P],
                            rhs=wo_sb[:, cc, nh * 512 : (nh + 1) * 512],
                            start=(cc == 0),
                            stop=(cc == 1),
                        )
                    dst = osb[:, nh * 512 : (nh + 1) * 512]
                    if use_act and nh == 1:
                        nc.scalar.copy(out=dst, in_=pso[:])
                    else:
                        nc.vector.tensor_copy(out=dst, in_=pso[:])
                nc.sync.dma_start(out=OUT[qc * P : (qc + 1) * P, :], in_=osb[:])

            bg_queue = []

            def drain_bg(k=1):
                for _ in range(k):
                    if bg_queue:
                        bg_queue.pop(0)()

            # ---------------- prefix: input panels + dc0 proj ----------------
            q_panels = {}
            k_panels = {}

            def load_panel(which, pan, eng=None):
                nbufs = 4 if (USE_FP8_QK and which == "k") else 2
                t = panpool.tile(
                    [P, KO, PAN],
                    BF16,
                    tag=f"{which}pan",
                    bufs=nbufs,
                    name=f"{which}p{pan}",
                )
                src = qt_r if which == "q" else kt_r
                (eng or nc.sync).dma_start(
                    out=t[:], in_=src[:, :, pan * PAN : (pan + 1) * PAN]
                )
                (q_panels if which == "q" else k_panels)[pan] = t

            def load_mask(kc, half):
                nc.sync.dma_start(
                    out=mT[:, kc, half * QG : (half + 1) * QG],
                    in_=mt_r[:, kc, half * QG : (half + 1) * QG],
                )

            # Minimal prefix: only what the first QK/PV of window (0,0) needs.
            # The rest of the dc0/V projections are woven into window 0 via
            # per-kc hooks, several iterations ahead of their consumers.
            bias_dmas()
            # scalar-AP operands (tensor_scalar scalar1) carry no dependency
            # edge; this DVE self-copy fences all later DVE evictions behind
            # the bias DMAs (DVE executes in order).
            nc.vector.tensor_copy(out=bk_sb[:], in_=bk_sb[:])
            nc.vector.tensor_copy(out=bq_sb[:], in_=bq_sb[:])
            load_panel("k", 0)
            load_panel("q", 0)
            load_panel("q", 1)
            proj_group("k", 0, 0)
            proj_group("q", 0, 0)
            load_panel("q", 2)
            proj_group("q", 1, 0)
            load_panel("k", 1)
            proj_group("q", 2, 0)
            load_panel("q", 3)
            late_weight_dmas()  # wv/wo
            proj_group("k", 1, 0)
            proj_group("q", 3, 0)
            v_group(0, 0)
            v_group(0, 1)
            load_panel("k", 2)
            if USE_FP8_QK:
                proj_group("k", 2, 0)
                load_panel("k", 3)
                proj_group("k", 3, 0)
                fold_fp8("q", (0, 1), 0, 1024)
                fold_fp8("k", (0, 1), 0, 1024)
            for kc in range(4):
                load_mask(kc, 0)

            # window-0 hooks: stream the remaining dc0 K / V projections and
            # the mask halves just ahead of their consumers. Ordering rule: a
            # panel load must be emitted after every consumer of the tile
            # previously occupying its pool slot.
            w0_hooks = {}

            def _add_hook(kc, fn):
                w0_hooks.setdefault(kc, []).append(fn)

            if not USE_FP8_QK:
                for j in range(6):  # K panels 2..7 at kc=2j, used kc=2j+4
                    _add_hook(2 * j, lambda p=j + 2: proj_group("k", p, 0))
                    _add_hook(2 * j + 1, lambda p=j + 1: v_group(p, 0))
                    _add_hook(2 * j + 1, lambda p=j + 1: v_group(p, 1))
                    if j < 5:  # panels 3..7; load after slot's last reader
                        _add_hook(2 * j + 1, lambda p=j + 3: load_panel("k", p))
                _add_hook(13, lambda: v_group(7, 0))
                _add_hook(13, lambda: v_group(7, 1))
            else:
                # K panels 4..7: load at odd hooks after the slot's V reader,
                # project 1 hook later; folds trail the evictions just ahead
                # of the QK kc that consumes them. V panels 0..7 stream at
                # odd hooks (PV trails by PV_DELAY).
                for j in range(4):
                    _add_hook(2 * j + 1, lambda p=j: v_group(p, 0))
                    _add_hook(2 * j + 1, lambda p=j: v_group(p, 1))
                    _add_hook(2 * j + 1, lambda p=j + 4: load_panel("k", p))
                for j in range(4):
                    _add_hook(2 * j + 2, lambda p=j + 4: proj_group("k", p, 0))
                _add_hook(6, lambda: fold_fp8("k", (0, 1), 1024, 1536))
                _add_hook(10, lambda: fold_fp8("k", (0, 1), 1536, 2048))
                _add_hook(11, lambda: fold_fp8("q", (0, 1), 1024, 2048))
            for i in range(4):  # q panels 4..7 for qg1
                _add_hook(2 * i + 1, lambda p=4 + i: load_panel("q", p))
                _add_hook(2 * i + 3, lambda p=4 + i: proj_group("q", p, 0))
            for kc in range(12):  # qg0 mask rows 4..15
                _add_hook(kc, lambda r=kc + 4: load_mask(r, 0))
            for kc in range(8, 16):  # qg1 mask halves during late window 0
                _add_hook(kc, lambda r=2 * (kc - 8): load_mask(r, 1))
                _add_hook(kc, lambda r=2 * (kc - 8) + 1: load_mask(r, 1))

            w1_hooks = {}
            if USE_FP8_QK:
                for j in range(4):  # V panels 4..7 spill into window (0,1)
                    w1_hooks.setdefault(2 * j + 1, []).append(
                        lambda p=j + 4: v_group(p, 0)
                    )
                    w1_hooks.setdefault(2 * j + 1, []).append(
                        lambda p=j + 4: v_group(p, 1)
                    )

            # background for windows (0,1)..(1,1): dc1 projections
            bg_queue.append(lambda: load_panel("k", 0))
            bg_queue.append(lambda: load_panel("q", 0))
            bg_queue.append(lambda: load_panel("k", 1))
            bg_queue.append(lambda: load_panel("q", 1))

            def dc1_unit(which, pan):
                proj_group(which, pan, 1)
                if pan + 2 < NPAN:
                    load_panel(which, pan + 2)

            for pan in range(NPAN):
                bg_queue.append(lambda p=pan: dc1_unit("k", p))
                bg_queue.append(lambda p=pan: dc1_unit("q", p))
                if USE_FP8_QK and pan == 3:
                    bg_queue.append(lambda: fold_fp8("k", (2, 3), 0, 1024))
                    bg_queue.append(lambda: fold_fp8("q", (2, 3), 0, 1024))
            if USE_FP8_QK:
                bg_queue.append(lambda: fold_fp8("k", (2, 3), 1024, 2048))
                bg_queue.append(lambda: fold_fp8("q", (2, 3), 1024, 2048))

            # ---------------- attention windows ----------------
            mask_ctr = [0]

            def window(h, qg, bg_per_kc, carry, hooks, pv_delay=12):
                dc, po = h // 2, (h % 2) * HD
                q0 = qg * QG
                pv = [
                    pvps.tile([P, 4, HD + 1], F32, tag="pv", name=f"pv{h}{qg}{i}")
                    for i in range(2)
                ]

                def pv_batch(kc):
                    # one accumulation group per PSUM bank: start once, stop
                    # once; per-element has_written handles first-write-vs-
                    # accumulate across the four qi sub-regions.
                    for qh in range(2):
                        for qi in range(4):
                            nc.tensor.matmul(
                                pv[qh][:, qi, :],
                                lhsT=mex[
                                    :, kc, (qh * 4 + qi) * P : (qh * 4 + qi + 1) * P
                                ],
                                rhs=v_sb[:, kc, h, :],
                                start=(kc == 0 and qi == 0),
                                stop=(kc == NKC - 1 and qi == 3),
                            )

                def norms_qh(qh):
                    def _n():
                        rc = rcpool.tile([P, 4], F32, tag="rc", name=f"rc{h}{qg}{qh}")
                        nc.vector.reciprocal(out=rc[:], in_=pv[qh][:, :, HD])
                        for qi in range(4):
                            qc = qg * 8 + qh * 4 + qi
                            nc.vector.tensor_scalar(
                                out=onorm[:, dc2(h), qc, hcol(h)],
                                in0=pv[qh][:, qi, 0:HD],
                                scalar1=rc[:, qi : qi + 1],
                                scalar2=None,
                                op0=ALU.mult,
                            )

                    return _n

                for kc in range(NKC):
                    if kc < len(carry):
                        carry[kc]()
                    for fn in hooks.get(kc, ()):
                        fn()
                    ps = qkps.tile([P, QG], F32, tag="qk", name=f"qk{h}{qg}{kc}")
                    for half in range(2):
                        hs = slice(half * 512, (half + 1) * 512)
                        if USE_FP8_QK:
                            nc.tensor.matmul(
                                ps[:, hs],
                                lhsT=k8[
                                    32 * h : 32 * h + 32, :, kc * P : (kc + 1) * P
                                ],
                                rhs=q8[
                                    32 * h : 32 * h + 32,
                                    :,
                                    q0 + half * 512 : q0 + (half + 1) * 512,
                                ],
                                start=True,
                                stop=True,
                                perf_mode=PM.DoubleRow,
                                tile_position=(32 * h, 0),
                            )
                        else:
                            nc.tensor.matmul(
                                ps[:, hs],
                                lhsT=kT[po : po + HD, dc, kc * P : (kc + 1) * P],
                                rhs=qT[
                                    po : po + HD,
                                    dc,
                                    q0 + half * 512 : q0 + (half + 1) * 512,
                                ],
                                start=True,
                                stop=True,
                            )
                    if kc in QUAD_KCS:
                        # exp(x) ~= ((E*(E + 4/S) * S^2/8) + 1)^2 on DVE
                        u = qpool.tile([P, QG], BF16, tag="u", name=f"u{h}{qg}{kc}")
                        t2 = qpool.tile([P, QG], BF16, tag="t2", name=f"t{h}{qg}{kc}")
                        nc.vector.scalar_tensor_tensor(
                            out=u[:],
                            in0=ps[:],
                            scalar=4.0 / SCALE,
                            in1=ps[:],
                            op0=ALU.add,
                            op1=ALU.mult,
                        )
                        nc.vector.tensor_scalar(
                            out=t2[:],
                            in0=u[:],
                            scalar1=SCALE * SCALE / 8.0,
                            scalar2=1.0,
                            op0=ALU.mult,
                            op1=ALU.add,
                        )
                        nc.vector.tensor_mul(out=mex[:, kc, :], in0=t2[:], in1=t2[:])
                    else:
                        nc.scalar.activation(
                            out=mex[:, kc, :], in_=ps[:], func=AF.Exp, scale=SCALE
                        )
                    mask_ctr[0] += 1
                    eng = (
                        nc.gpsimd
                        if (POOL_MASK_EVERY and mask_ctr[0] % POOL_MASK_EVERY == 0)
                        else nc.vector
                    )
                    eng.tensor_mul(
                        out=mex[:, kc, :],
                        in0=mex[:, kc, :],
                        in1=mT[:, kc, q0 : q0 + QG],
                    )
                    # PV trails deep: mex stays valid until the next
                    # window's ACT overwrites it, so late PV batches spill
                    # into the next window's first iterations via the carry.
                    if kc >= pv_delay:
                        pv_batch(kc - pv_delay)
                    if bg_per_kc and kc % 2 == 1:
                        drain_bg(bg_per_kc)
                # leftover PV batches + normalization are carried into
                # the next window's iterations, one closure per kc
                pv_left = [
                    (lambda kc=kc: pv_batch(kc))
                    for kc in range(NKC - pv_delay, NKC)
                ]
                return {"pv_left": pv_left, "norms": [norms_qh(0), norms_qh(1)]}

            def dc2(h):
                return h // 2

            def hcol(h):
                return slice((h % 2) * HD, (h % 2) * HD + HD)

            def transpose_qg(qg, dc):
                # batched xbar transpose: out[p, j, f] = in[f, j*128 + p]
                nc.sync.dma_start_transpose(
                    out=oT[:, dc, qg * QG : (qg + 1) * QG].rearrange(
                        "p (j f) -> p j f", j=8
                    ),
                    in_=onorm[:, dc, qg * 8 : (qg + 1) * 8, :],
                )

            def transpose_half(qg, dc, qh):
                q0 = qg * QG + qh * 512
                nc.sync.dma_start_transpose(
                    out=oT[:, dc, q0 : q0 + 512].rearrange("p (j f) -> p j f", j=4),
                    in_=onorm[:, dc, qg * 8 + qh * 4 : qg * 8 + qh * 4 + 4, :],
                )

            last_carry = [None]

            carry = []

            def run_window(h, qg, bg_per_kc, hooks=None, pv_delay=12):
                nonlocal carry
                c = window(h, qg, bg_per_kc, carry, hooks or {}, pv_delay)
                last_carry[0] = c

                def _norms_both():
                    c["norms"][0]()
                    c["norms"][1]()

                carry = c["pv_left"] + [_norms_both]

            # pair 0 (heads 0,1); window 0 carries the rest of the dc0/V
            # projections via hooks, later windows drain dc1 projections
            run_window(0, 0, 0, hooks=w0_hooks)
            run_window(0, 1, 1, hooks=w1_hooks)
            run_window(1, 0, 1)
            run_window(1, 1, 1)
            assert not bg_queue, f"{len(bg_queue)} undrained bg units"
            # pair 1 (heads 2,3): out-proj trails head 3
            run_window(2, 0, 0)
            transpose_qg(0, 0)
            transpose_qg(1, 0)
            run_window(2, 1, 0)
            run_window(3, 0, 0)
            run_window(3, 1, 0, hooks={13: [lambda: transpose_qg(0, 1)]})
            c = last_carry[0]
            for fn in c["pv_left"]:
                fn()
            c["norms"][0]()
            transpose_half(1, 1, 0)
            for qc in range(0, 4):
                out_proj(qc)
            c["norms"][1]()
            transpose_half(1, 1, 1)
            for qc in range(4, 12):
                out_proj(qc, use_act=(qc >= 8))
            for qc in range(12, 16):
                out_proj(qc, use_act=True)

    nc.finalize()
    return nc


_NC = None


def _get_nc():
    global _NC
    if _NC is None:
        _NC = build_nc()
    return _NC


def make_in_maps(Q, K, mask, Wq, bq, Wk, bk, Wv, bv, Wo, bo):
    Q = np.asarray(Q, np.float32)
    K = np.asarray(K, np.float32)
    mask = np.asarray(mask)
    Wq = np.asarray(Wq, np.float32)
    Wk = np.asarray(Wk, np.float32)
    Wv = np.asarray(Wv, np.float32)
    Wo = np.asarray(Wo, np.float32)
    qt = [np.ascontiguousarray(Q[b].T).astype(ml_dtypes.bfloat16) for b in range(B)]
    kt = [np.ascontiguousarray(K[b].T).astype(ml_dtypes.bfloat16) for b in range(B)]
    mt = [
        np.ascontiguousarray(mask[b].T).astype(ml_dtypes.bfloat16) for b in range(B)
    ]
    in_maps = []
    for c in range(8):
        b, hg = divmod(c, 4)
        cols = slice(hg * DL, (hg + 1) * DL)
        in_maps.append(
            {
                "qt": qt[b],
                "kt": kt[b],
                "mt": mt[b],
                "wqt": np.ascontiguousarray(Wq[cols, :].T).astype(ml_dtypes.bfloat16),
                "wkt": np.ascontiguousarray(Wk[cols, :].T).astype(ml_dtypes.bfloat16),
                "wvt": np.ascontiguousarray(Wv[cols, :].T).astype(ml_dtypes.bfloat16),
                "wot": np.ascontiguousarray(Wo[:, cols].T).astype(ml_dtypes.bfloat16),
                "bq": np.ascontiguousarray(np.asarray(bq, np.float32)[cols]),
                "bk": np.ascontiguousarray(np.asarray(bk, np.float32)[cols]),
                "bv": np.ascontiguousarray(np.asarray(bv, np.float32)[cols]),
            }
        )
    return in_maps


def assemble(results, bo):
    O = np.zeros((B, N, D), np.float32)
    for c in range(8):
        b = c // 4
        O[b] += results[c]["out"]
    O += np.asarray(bo, np.float32)[None, None, :]
    return O


def kernel(Q, K, mask, Wq, bq, Wk, bk, Wv, bv, Wo, bo):
    nc = _get_nc()
    in_maps = make_in_maps(Q, K, mask, Wq, bq, Wk, bk, Wv, bv, Wo, bo)
    res = run_bass_kernel_spmd(nc, in_maps, core_ids=list(range(8)))
    return assemble(res.results, bo)
